# revision 24
# baseline (speedup 1.0000x reference)
"""Trainium2 Bass kernel for nn_Cross_attention_multi (sparse_attention).

Pipeline (8 NeuronCores, SPMD, one NEFF):
  Stage A  - 3D conv (SAME, 3x3x3) spatially sharded: each core convolves a
             6-row h-strip for all 32 channels of both x and y. bf16 matmuls
             with K=97 (kh x ci + bias row) and M=128 (4 output d-slices x
             32 channels) accumulate 3 kw-taps per input d-slab into f32
             PSUM; the kd tap is absorbed into the M-packing. Inputs arrive
             kh-pre-expanded from the host so the slab loads are single
             contiguous DMAs. Each PSUM block drains with ONE full-lane
             [128, 288] copy; the pd-scatter rides in the staging DMA AP.
  AllToAll - split into NSPLIT sub-collectives per tensor over d-groups so
             redistribution pipelines with the conv (x) and with attention
             stripes (y). Buffers are [dst_core, c_lo, d, pw, v] so staging
             writes 576B runs and the gather reads 5KB contiguous runs.
  Stage B  - res_trans = ONE matmul (Weff = W2@W1 fused on host; leaky has
             no inner nonlinearity) + native Lrelu on ACT. Attention runs in
             m-stripes per y d-group: att[:, :, stripe] = aTx^T @ aTy_chunk
             in [128, 256] PSUM tiles, drained ACT/DVE, streamed to HBM bf16.
"""

import sys

sys.path.insert(0, "/opt/trn_rl_repo")

import contextlib

import numpy as np
import ml_dtypes

import concourse.bass as bass
import concourse.bacc as bacc
import concourse.mybir as mybir
import concourse.tile as tile
from concourse import bass_utils

N_CORES = 8
C, D, H, W = 32, 36, 48, 48
P = 9
L = 1024
F32 = mybir.dt.float32
BF16 = mybir.dt.bfloat16
BF_NP = ml_dtypes.bfloat16

NSPLIT = 2          # sub-collectives per tensor
GD = D // NSPLIT    # d's per group
LDG = 4 // NSPLIT   # ld's per group


def build_program(n_iters=1, phases="abc", unroll=False):
    nc = bacc.Bacc(
        "TRN2", target_bir_lowering=False, debug=False, num_devices=N_CORES
    )

    # kh-expanded strips: row 32*kh+c = input rows [kh, kh+6) of the 8-row
    # halo strip; row 96 = ones (feeds the bias row of lw).
    xs3 = nc.dram_tensor("xs3", [97, D, 6, 50], BF16, kind="ExternalInput")
    ys3 = nc.dram_tensor("ys3", [97, D, 6, 50], BF16, kind="ExternalInput")
    # [rel_d(6), kw(3), kh*32+ci (96) + bias(1), 32*dd+co (128)]
    lwx = nc.dram_tensor("lwx", [6, 3, 97, 128], BF16, kind="ExternalInput")
    lwy = nc.dram_tensor("lwy", [6, 3, 97, 128], BF16, kind="ExternalInput")
    # weff[p, o] = ((W2 @ W1) / 9)[o, p]
    weff = nc.dram_tensor("weff", [81, 81], BF16, kind="ExternalInput")
    att = nc.dram_tensor("att", [4, L, L], BF16, kind="ExternalOutput")

    Copy = mybir.ActivationFunctionType.Copy
    Ident = mybir.ActivationFunctionType.Identity
    amax = mybir.AluOpType.max

    with tile.TileContext(nc) as tc:
        with tc.tile_pool(name="dram", bufs=1, space="DRAM") as dram:
            # [dst core, c_lo, d within group, pw, v=lhw_local]
            ai = [
                [
                    dram.tile([N_CORES, 4, GD, 9, 32], BF16, name=f"ai{t}g{g}")
                    for g in range(NSPLIT)
                ]
                for t in range(2)
            ]
            ao = [
                [
                    dram.tile([N_CORES, 4, GD, 9, 32], BF16, name=f"ao{t}g{g}")
                    for g in range(NSPLIT)
                ]
                for t in range(2)
            ]

            def collective(tt, g):
                nc.gpsimd.collective_compute(
                    "AllToAll",
                    mybir.AluOpType.bypass,
                    replica_groups=[list(range(N_CORES))],
                    ins=[ai[tt][g].opt()],
                    outs=[ao[tt][g].opt()],
                )

            def emit_body():
                # ------------ Stage A: conv + split AllToAlls ------------
                if "a" in phases:
                    with (
                        tc.tile_pool(name="slab", bufs=2) as slab_pool,
                        tc.tile_pool(name="wts", bufs=1) as wts_pool,
                        tc.tile_pool(name="stageA", bufs=3) as stage_pool,
                        tc.tile_pool(name="psumA", bufs=6, space="PSUM") as psumA,
                    ):
                        # prefetch both tensors (contiguous, d-chunked)
                        lws, slabs = [], []
                        for tt, (src, lw_d) in enumerate(
                            [(xs3, lwx), (ys3, lwy)]
                        ):
                            lw = wts_pool.tile([97, 18, 128], BF16, tag=f"lw{tt}")
                            nc.sync.dma_start(
                                lw[:].rearrange("p (r k) m -> p r k m", k=3),
                                lw_d[:].transpose([2, 0, 1, 3]),
                            )
                            s_all = slab_pool.tile(
                                [97, D, 6, 50], BF16, tag="slab"
                            )
                            for dc in range(3):
                                dsl = slice(12 * dc, 12 * dc + 12)
                                nc.sync.dma_start(
                                    s_all[:, dsl], src[:, dsl]
                                )
                            lws.append(lw)
                            slabs.append(s_all)

                        for tt in range(2):
                            lw, s_all = lws[tt], slabs[tt]
                            for b in range(9):
                                rels = [
                                    r for r in range(6) if 0 <= 4 * b + r - 1 < D
                                ]
                                pt = psumA.tile([128, 288], F32, tag="pa")
                                n_mm = 3 * len(rels)
                                i = 0
                                for rel in rels:
                                    din = 4 * b + rel - 1
                                    for kw in range(3):
                                        nc.tensor.matmul(
                                            pt[:],
                                            lw[:, 3 * rel + kw, :],
                                            s_all[:, din, :, kw : kw + 48],
                                            start=(i == 0),
                                            stop=(i == n_mm - 1),
                                        )
                                        i += 1
                                # ONE full-lane drain per block:
                                # [128=(dd,co), 32lhw, 9pw]->[128, 9pw, 32lhw]
                                st4 = stage_pool.tile(
                                    [128, 9, 32], BF16, tag="st4"
                                )
                                srcp = pt[:].rearrange("p (l w) -> p w l", w=9)
                                if b % 2 == 0:
                                    nc.scalar.activation(st4[:], srcp, Copy)
                                else:
                                    nc.vector.tensor_copy(st4[:], srcp)
                                # staging DMA per d-group segment; partition
                                # (dd, co=(i,c)) row -> ai[g][i, c, dg, pw, v]
                                d0 = 4 * b
                                cuts = [0]
                                for dd in range(1, 4):
                                    if (d0 + dd) % GD == 0:
                                        cuts.append(dd)
                                cuts.append(4)
                                for s0, s1 in zip(cuts[:-1], cuts[1:]):
                                    g = (d0 + s0) // GD
                                    dg = (d0 + s0) % GD
                                    ndd = s1 - s0
                                    nc.sync.dma_start(
                                        ai[tt][g][
                                            :, :, dg : dg + ndd
                                        ].rearrange(
                                            "i c e w v -> e i c w v"
                                        ),
                                        st4[32 * s0 : 32 * s1],
                                    )
                                # issue a group's collective right after the
                                # block staging its last d
                                if "c" in phases:
                                    for g in range(NSPLIT):
                                        if d0 <= GD * (g + 1) - 1 <= d0 + 3:
                                            collective(tt, g)

                # collectives alone (for phase timing): phases="c"
                if "c" in phases and "a" not in phases:
                    for tt in range(2):
                        for g in range(NSPLIT):
                            collective(tt, g)

                # ---------------- Stage B ----------------
                if "b" in phases:
                    with (
                        tc.tile_pool(name="wtsB", bufs=1) as wtsB,
                        tc.tile_pool(name="sbX", bufs=1) as sbX,
                        tc.tile_pool(name="sbY", bufs=8) as sbY,
                        tc.tile_pool(name="attst", bufs=6) as attst_pool,
                        tc.tile_pool(name="psumX", bufs=2, space="PSUM") as psumX,
                        tc.tile_pool(name="psumV", bufs=2, space="PSUM") as psumV,
                        tc.tile_pool(name="psumT", bufs=4, space="PSUM") as psumT,
                    ):
                        weff_sb = wtsB.tile([81, 81], BF16, tag="weff")
                        nc.scalar.dma_start(weff_sb[:], weff[:])

                        def gather(tt, c_lo, ld, dst, q):
                            # dst [81, 256] <- ao[tt][g][:, c_lo, 9lg:9lg+9].
                            # 64B-run reads (j-stride is unavoidable); spread
                            # across both HWDGE queues.
                            g, lg = ld // LDG, ld % LDG
                            eng = nc.sync if q == 0 else nc.scalar
                            eng.dma_start(
                                dst.rearrange("p (j v) -> p j v", j=8),
                                ao[tt][g][
                                    :, c_lo, 9 * lg : 9 * lg + 9
                                ].rearrange("j e w v -> (e w) j v"),
                            )

                        # x side: gathers split across queues, then rt
                        tTx = []
                        for c_lo in range(4):
                            tT = sbX.tile([81, L], BF16, tag=f"tTx{c_lo}")
                            for ld in range(4):
                                gather(
                                    0, c_lo, ld,
                                    tT[:, 256 * ld : 256 * ld + 256],
                                    (c_lo + ld) % 2,
                                )
                            tTx.append(tT)
                        # leaky(v) = max(0.2v, v): the HW Lrelu alpha operand
                        # is silently ignored (always 0.01), so ACT scales and
                        # DVE maxes.
                        def leaky(dst, v, tmp_tag, n):
                            t1 = sbY.tile([81, n], BF16, tag=tmp_tag)
                            nc.scalar.activation(
                                t1[:], v[:], Ident, scale=0.2
                            )
                            nc.vector.tensor_tensor(dst, t1[:], v[:], amax)

                        aTx = []
                        for c_lo in range(4):
                            a_sb = sbX.tile([81, L], BF16, tag=f"aTx{c_lo}")
                            for nch in range(2):
                                sl = slice(512 * nch, 512 * nch + 512)
                                v = psumX.tile([81, 512], F32, tag="vx")
                                nc.tensor.matmul(
                                    v[:], weff_sb[:], tTx[c_lo][:, sl],
                                    start=True, stop=True,
                                )
                                leaky(a_sb[:, sl], v, "t1x", 512)
                            aTx.append(a_sb)

                        # y side: m-striped attention; stripe ld+1's gathers
                        # are emitted before stripe ld's output DMAs so they
                        # never queue behind 4 MB of writes.
                        tTy = {}

                        def gather_stripe(ld):
                            for c_lo in range(4):
                                t = sbY.tile([81, 256], BF16, tag="tTy")
                                gather(1, c_lo, ld, t[:], (c_lo + ld) % 2)
                                tTy[(ld, c_lo)] = t

                        gather_stripe(0)
                        seq = [
                            (ld, c_lo) for ld in range(4) for c_lo in range(4)
                        ]
                        aYcs = {}

                        def rt_emit(k):
                            # one step ahead of att so PE/ACT never serialize
                            # on the leaky of the CURRENT stripe-channel
                            ld, c_lo = seq[k]
                            v = psumV.tile([81, 256], F32, tag="vy")
                            nc.tensor.matmul(
                                v[:], weff_sb[:], tTy.pop((ld, c_lo))[:],
                                start=True, stop=True,
                            )
                            aYc = sbY.tile([81, 256], BF16, tag="aTy")
                            leaky(aYc[:], v, "t1y", 256)
                            aYcs[k] = aYc

                        rt_emit(0)
                        for k, (ld, c_lo) in enumerate(seq):
                            if c_lo == 0 and ld + 1 < 4:
                                gather_stripe(ld + 1)
                            if k + 1 < len(seq):
                                rt_emit(k + 1)
                            aYc = aYcs.pop(k)
                            for lc in range(8):
                                pa = psumT.tile([128, 256], F32, tag="pt")
                                nc.tensor.matmul(
                                    pa[:],
                                    aTx[c_lo][:, 128 * lc : 128 * lc + 128],
                                    aYc[:],
                                    start=True, stop=True,
                                )
                                stt = attst_pool.tile(
                                    [128, 256], BF16, tag="stt"
                                )
                                # drains 5 ACT / 3 DVE: DVE also runs the
                                # leaky maxes
                                if lc in (1, 4, 7):
                                    nc.vector.tensor_copy(stt[:], pa[:])
                                else:
                                    nc.scalar.activation(stt[:], pa[:], Copy)
                                eng = nc.sync if (lc + c_lo) % 2 else nc.scalar
                                eng.dma_start(
                                    att[
                                        c_lo,
                                        128 * lc : 128 * lc + 128,
                                        256 * ld : 256 * ld + 256,
                                    ],
                                    stt[:],
                                )

            if n_iters > 1 and not unroll:
                with tc.For_i(0, n_iters):
                    emit_body()
            else:
                for _it in range(n_iters):
                    if _it:
                        tc.strict_bb_all_engine_barrier()
                    emit_body()

    nc.compile()
    return nc


def host_inputs(x, y, Wx, bx, Wy, by, W1, W2):
    x = np.asarray(x, np.float32)
    y = np.asarray(y, np.float32)
    Wx = np.asarray(Wx, np.float32)
    bx = np.asarray(bx, np.float32)
    Wy = np.asarray(Wy, np.float32)
    by = np.asarray(by, np.float32)
    W1 = np.asarray(W1, np.float32)
    W2 = np.asarray(W2, np.float32)

    def strips3(x0):
        # kh-expanded halo strips + ones row: [97, D, 6, 50]
        out = []
        for j in range(N_CORES):
            s = np.zeros((C, D, 8, 50), np.float32)
            r0, r1 = max(0, 6 * j - 1), min(48, 6 * j + 7)
            d0 = r0 - (6 * j - 1)
            s[:, :, d0 : d0 + (r1 - r0), 1:49] = x0[:, :, r0:r1, :]
            s3 = np.empty((97, D, 6, 50), np.float32)
            for kh in range(3):
                s3[32 * kh : 32 * kh + 32] = s[:, :, kh : kh + 6, :]
            s3[96] = 1.0
            out.append(s3.astype(BF_NP))
        return out

    def make_lw(Wc, bc):
        # lw[rel, kw, kh*32+ci, 32*dd+co] = Wc[co, ci, rel-dd, kh, kw];
        # row 96 carries the bias (fed by the ones row) at kd=1, kw=1.
        lw = np.zeros((6, 3, 97, 128), np.float32)
        for rel in range(6):
            for dd in range(4):
                kd = rel - dd
                if 0 <= kd < 3:
                    blk = np.transpose(Wc[:, :, kd], (3, 2, 1, 0)).reshape(
                        3, 96, 32
                    )
                    lw[rel, :, :96, 32 * dd : 32 * dd + 32] = blk
                if kd == 1:
                    lw[rel, 1, 96, 32 * dd : 32 * dd + 32] = bc
        return lw.astype(BF_NP)

    xs_l, ys_l = strips3(x[0]), strips3(y[0])
    common = {
        "lwx": make_lw(Wx, bx),
        "lwy": make_lw(Wy, by),
        # res_trans = leaky(W2 @ (W1 @ t)) = leaky(Weff @ t); /9 per side
        # gives the att /81 overall (leaky is positively homogeneous).
        "weff": np.ascontiguousarray((W2 @ W1 / 9.0).T).astype(BF_NP),
    }
    return [
        {"xs3": xs_l[j], "ys3": ys_l[j], **common} for j in range(N_CORES)
    ]


_CACHED_NC = None


def get_program():
    global _CACHED_NC
    if _CACHED_NC is None:
        _CACHED_NC = build_program()
    return _CACHED_NC


def _probe_device():
    """Absorb a wedged-worker state left by a previous process: the first
    device op after a wedge fails and resets the worker; a retry succeeds."""
    import time

    import jax

    for _ in range(3):
        try:
            jax.block_until_ready(
                jax.jit(lambda a: a + 1)(np.zeros(8, np.float32))
            )
            return
        except Exception:
            time.sleep(2)


def kernel(x, y, Wx, bx, Wy, by, W1, W2):
    import time

    nc = get_program()
    in_maps = host_inputs(x, y, Wx, bx, Wy, by, W1, W2)
    _probe_device()
    last = None
    for _ in range(2):
        try:
            res = bass_utils.run_bass_kernel_spmd(
                nc, in_maps, core_ids=list(range(N_CORES))
            )
            break
        except Exception as e:
            last = e
            time.sleep(2)
    else:
        raise last
    out = np.concatenate([r["att"] for r in res.results], axis=0)[None]
    return out.astype(np.float32)


# revision 28
# speedup vs baseline: 1.2607x; 1.2607x over previous
"""Trainium2 Bass kernel for nn_Cross_attention_multi (sparse_attention).

Pipeline (8 NeuronCores, SPMD, one NEFF):
  Stage A  - 3D conv (SAME, 3x3x3) spatially sharded: each core convolves a
             6-row h-strip for all 32 channels of both x and y. bf16 matmuls
             with K=97 (kh x ci + bias row) and M=128 (4 output d-slices x
             32 channels) accumulate 3 kw-taps per input d-slab into f32
             PSUM. Inputs arrive kh-pre-expanded AND d-chunk-contiguous from
             the host (DRAM-read throughput collapses ~10x when the read
             streams are strided). One full-lane [128, 288] PSUM drain per
             block; the pd-scatter rides in the staging DMA AP (576B runs).
  AllToAll - one collective per tensor (per-call cost ~7us, so no splitting);
             x's overlaps y's conv. Buffers are [dst_core, c_lo, d, pw, v].
  Stage B  - res_trans = ONE matmul (Weff = W2@W1 fused on host) + leaky as
             ACT-scale + DVE-max (the HW Lrelu alpha operand is ignored).
             All stage-B matmuls pad the 81-long patch contraction to K=96
             (K not a multiple of 32 runs ~2.2x slower); the zero weight
             rows make the pad contribution exactly 0. Attention in
             [128, 512] PSUM tiles with deep pool rotation, drained
             ACT(9)/DVE(7), streamed to HBM bf16 on the scalar queue while
             gathers run on sync.
"""

import sys

sys.path.insert(0, "/opt/trn_rl_repo")

import contextlib

import numpy as np
import ml_dtypes

import concourse.bass as bass
import concourse.bacc as bacc
import concourse.mybir as mybir
import concourse.tile as tile
from concourse import bass_utils

N_CORES = 8
C, D, H, W = 32, 36, 48, 48
P = 9
L = 1024
F32 = mybir.dt.float32
BF16 = mybir.dt.bfloat16
BF_NP = ml_dtypes.bfloat16


def build_program(n_iters=1, phases="abc", unroll=False):
    nc = bacc.Bacc(
        "TRN2", target_bir_lowering=False, debug=False, num_devices=N_CORES
    )

    # [chunk, 97, 12*300]: kh-expanded strips (row 32*kh+c = rows [kh, kh+6)
    # of the 8-row halo strip; row 96 = ones feeding the bias row of lw),
    # flattened so each chunk is one fully contiguous DRAM read.
    xs3 = nc.dram_tensor("xs3", [3, 97, 3600], BF16, kind="ExternalInput")
    ys3 = nc.dram_tensor("ys3", [3, 97, 3600], BF16, kind="ExternalInput")
    # [rel_d(6), kw(3), kh*32+ci (96) + bias(1), 32*dd+co (128)]
    lwx = nc.dram_tensor("lwx", [6, 3, 97, 128], BF16, kind="ExternalInput")
    lwy = nc.dram_tensor("lwy", [6, 3, 97, 128], BF16, kind="ExternalInput")
    # weffp[k=96 (81 real + 15 zero), o=81] = padded ((W2 @ W1) / 9).T
    weffp = nc.dram_tensor("weffp", [96, 81], BF16, kind="ExternalInput")
    att = nc.dram_tensor("att", [4, L, L], BF16, kind="ExternalOutput")

    Copy = mybir.ActivationFunctionType.Copy
    Ident = mybir.ActivationFunctionType.Identity
    amax = mybir.AluOpType.max

    with tile.TileContext(nc) as tc:
        with (
            tc.tile_pool(name="dram", bufs=1, space="DRAM") as dram,
            # stage-B SBUF lives outside the timing loop so the K=96 pad
            # rows can be zeroed exactly once
            tc.tile_pool(name="wtsB", bufs=1) as wtsB,
            tc.tile_pool(name="sbB", bufs=1) as sbB,
            tc.tile_pool(name="sbT", bufs=4) as sbT,
            tc.tile_pool(name="attst", bufs=4) as attst_pool,
        ):
            # [dst core, c_lo, d, pw, v=lhw_local] per tensor
            ai = [
                dram.tile([N_CORES, 4, D, 9, 32], BF16, name=f"ai{t}")
                for t in range(2)
            ]
            ao = [
                dram.tile([N_CORES, 4, D, 9, 32], BF16, name=f"ao{t}")
                for t in range(2)
            ]

            weff_sb = wtsB.tile([96, 81], BF16, tag="weff")
            nc.scalar.dma_start(weff_sb[:], weffp[:])

            # persistent stage-B tiles; pad rows zeroed once
            tTx, tTy, aTx, aTy = [], [], [], []
            for c_lo in range(4):
                tTx.append(
                    sbB.tile([96, L], BF16, tag=f"tTx{c_lo}", name=f"tTx{c_lo}")
                )
                tTy.append(
                    sbB.tile([96, L], BF16, tag=f"tTy{c_lo}", name=f"tTy{c_lo}")
                )
                aTx.append(
                    sbB.tile([96, L], BF16, tag=f"aTx{c_lo}", name=f"aTx{c_lo}")
                )
                aTy.append(
                    sbB.tile([96, L], BF16, tag=f"aTy{c_lo}", name=f"aTy{c_lo}")
                )
            for t in tTx + tTy + aTx + aTy:
                nc.vector.memset(t[:], 0.0)

            def collective(tt):
                nc.gpsimd.collective_compute(
                    "AllToAll",
                    mybir.AluOpType.bypass,
                    replica_groups=[list(range(N_CORES))],
                    ins=[ai[tt].opt()],
                    outs=[ao[tt].opt()],
                )

            def emit_body():
                # ------------ Stage A: conv + AllToAll per tensor ---------
                if "a" in phases:
                    with (
                        tc.tile_pool(name="slab", bufs=2) as slab_pool,
                        tc.tile_pool(name="wts", bufs=1) as wts_pool,
                        tc.tile_pool(name="stageA", bufs=3) as stage_pool,
                        tc.tile_pool(name="psumA", bufs=6, space="PSUM") as psumA,
                    ):
                        lws, slabs = [], []
                        for tt, (src, lw_d) in enumerate(
                            [(xs3, lwx), (ys3, lwy)]
                        ):
                            # x's loads on sync, y's on scalar: both tensors
                            # stream in parallel during x's conv
                            eng = nc.sync if tt == 0 else nc.scalar
                            lw = wts_pool.tile([97, 18, 128], BF16, tag=f"lw{tt}")
                            eng.dma_start(
                                lw[:].rearrange("p (r k) m -> p r k m", k=3),
                                lw_d[:].transpose([2, 0, 1, 3]),
                            )
                            s_all = slab_pool.tile(
                                [97, D, 6, 50], BF16, tag="slab"
                            )
                            for dc in range(3):
                                eng.dma_start(
                                    s_all[:, 12 * dc : 12 * dc + 12].rearrange(
                                        "p e h w -> p (e h w)"
                                    ),
                                    src[dc],
                                )
                            lws.append(lw)
                            slabs.append(s_all)

                        for tt in range(2):
                            lw, s_all = lws[tt], slabs[tt]
                            for b in range(9):
                                rels = [
                                    r for r in range(6) if 0 <= 4 * b + r - 1 < D
                                ]
                                pt = psumA.tile([128, 288], F32, tag="pa")
                                n_mm = 3 * len(rels)
                                i = 0
                                for rel in rels:
                                    din = 4 * b + rel - 1
                                    for kw in range(3):
                                        nc.tensor.matmul(
                                            pt[:],
                                            lw[:, 3 * rel + kw, :],
                                            s_all[:, din, :, kw : kw + 48],
                                            start=(i == 0),
                                            stop=(i == n_mm - 1),
                                        )
                                        i += 1
                                # ONE full-lane drain per block:
                                # [128=(dd,co), 32lhw, 9pw]->[128, 9pw, 32lhw]
                                st4 = stage_pool.tile(
                                    [128, 9, 32], BF16, tag="st4"
                                )
                                srcp = pt[:].rearrange("p (l w) -> p w l", w=9)
                                if b % 2 == 0:
                                    nc.scalar.activation(st4[:], srcp, Copy)
                                else:
                                    nc.vector.tensor_copy(st4[:], srcp)
                                # partition (dd, co=(i,c)) -> ai[i, c, d, pw, v]
                                nc.sync.dma_start(
                                    ai[tt][
                                        :, :, 4 * b : 4 * b + 4
                                    ].rearrange("i c e w v -> e i c w v"),
                                    st4[:],
                                )
                            if "c" in phases:
                                collective(tt)

                # collectives alone (for phase timing): phases="c"
                if "c" in phases and "a" not in phases:
                    for tt in range(2):
                        collective(tt)

                # ---------------- Stage B ----------------
                if "b" in phases:
                    with (
                        tc.tile_pool(name="psumX", bufs=3, space="PSUM") as psumX,
                        tc.tile_pool(name="psumT", bufs=5, space="PSUM") as psumT,
                    ):
                        def gather(tt, c_lo, ld, q):
                            # [81, 256] <- ao[tt][:, c_lo, 9ld:9ld+9]; 64B-run
                            # reads (j-stride is unavoidable)
                            dstt = (tTx if tt == 0 else tTy)[c_lo]
                            dst = dstt[:81, 256 * ld : 256 * ld + 256]
                            eng = nc.sync if q == 0 else nc.scalar
                            eng.dma_start(
                                dst.rearrange("p (j v) -> p j v", j=8),
                                ao[tt][
                                    :, c_lo, 9 * ld : 9 * ld + 9
                                ].rearrange("j e w v -> (e w) j v"),
                            )

                        # x gathers: emitted first, run during y's conv
                        for c_lo in range(4):
                            for ld in range(4):
                                gather(0, c_lo, ld, (c_lo + ld) % 2)

                        def leaky(dst, v, tag, n):
                            t1 = sbT.tile([81, n], BF16, tag=tag)
                            nc.scalar.activation(t1[:], v[:], Ident, scale=0.2)
                            nc.vector.tensor_tensor(dst, t1[:], v[:], amax)

                        def rt_emit(c_lo):
                            # y gathers for c_lo+1 go ahead of this channel's
                            # attention output traffic
                            for nch in range(2):
                                for tt in range(2):
                                    tT = (tTx, tTy)[tt][c_lo]
                                    a_sb = (aTx, aTy)[tt][c_lo]
                                    sl = slice(512 * nch, 512 * nch + 512)
                                    v = psumX.tile([81, 512], F32, tag="vx")
                                    nc.tensor.matmul(
                                        v[:], weff_sb[:], tT[:, sl],
                                        start=True, stop=True,
                                    )
                                    leaky(a_sb[:81, sl], v, f"t1_{tt}", 512)

                        for c_lo in range(4):
                            for ld in range(4):
                                gather(1, c_lo, ld, (c_lo + ld) % 2)
                        rt_emit(0)
                        for c_lo in range(4):
                            if c_lo + 1 < 4:
                                rt_emit(c_lo + 1)
                            for lc in range(8):
                                st = attst_pool.tile([128, L], BF16, tag="st")
                                for nch in range(2):
                                    pa = psumT.tile([128, 512], F32, tag="pt")
                                    nc.tensor.matmul(
                                        pa[:],
                                        aTx[c_lo][:, 128 * lc : 128 * lc + 128],
                                        aTy[c_lo][:, 512 * nch : 512 * nch + 512],
                                        start=True, stop=True,
                                    )
                                    dst = st[:, 512 * nch : 512 * nch + 512]
                                    # ACT 9 : DVE 7 (DVE also runs the maxes)
                                    k = 2 * lc + nch
                                    if k in (1, 3, 5, 7, 9, 11, 13):
                                        nc.vector.tensor_copy(dst, pa[:])
                                    else:
                                        nc.scalar.activation(dst, pa[:], Copy)
                                nc.scalar.dma_start(
                                    att[c_lo, 128 * lc : 128 * lc + 128, :],
                                    st[:],
                                )

            if n_iters > 1 and not unroll:
                with tc.For_i(0, n_iters):
                    emit_body()
            else:
                for _it in range(n_iters):
                    if _it:
                        tc.strict_bb_all_engine_barrier()
                    emit_body()

    nc.compile()
    return nc


def host_inputs(x, y, Wx, bx, Wy, by, W1, W2):
    x = np.asarray(x, np.float32)
    y = np.asarray(y, np.float32)
    Wx = np.asarray(Wx, np.float32)
    bx = np.asarray(bx, np.float32)
    Wy = np.asarray(Wy, np.float32)
    by = np.asarray(by, np.float32)
    W1 = np.asarray(W1, np.float32)
    W2 = np.asarray(W2, np.float32)

    def strips3(x0):
        # kh-expanded halo strips + ones row, d-chunked contiguous:
        # [3, 97, 3600]
        out = []
        for j in range(N_CORES):
            s = np.zeros((C, D, 8, 50), np.float32)
            r0, r1 = max(0, 6 * j - 1), min(48, 6 * j + 7)
            d0 = r0 - (6 * j - 1)
            s[:, :, d0 : d0 + (r1 - r0), 1:49] = x0[:, :, r0:r1, :]
            s3 = np.empty((97, D, 6, 50), np.float32)
            for kh in range(3):
                s3[32 * kh : 32 * kh + 32] = s[:, :, kh : kh + 6, :]
            s3[96] = 1.0
            s3 = s3.reshape(97, 3, 3600).transpose(1, 0, 2)
            out.append(np.ascontiguousarray(s3).astype(BF_NP))
        return out

    def make_lw(Wc, bc):
        # lw[rel, kw, kh*32+ci, 32*dd+co] = Wc[co, ci, rel-dd, kh, kw];
        # row 96 carries the bias (fed by the ones row) at kd=1, kw=1.
        lw = np.zeros((6, 3, 97, 128), np.float32)
        for rel in range(6):
            for dd in range(4):
                kd = rel - dd
                if 0 <= kd < 3:
                    blk = np.transpose(Wc[:, :, kd], (3, 2, 1, 0)).reshape(
                        3, 96, 32
                    )
                    lw[rel, :, :96, 32 * dd : 32 * dd + 32] = blk
                if kd == 1:
                    lw[rel, 1, 96, 32 * dd : 32 * dd + 32] = bc
        return lw.astype(BF_NP)

    xs_l, ys_l = strips3(x[0]), strips3(y[0])
    weffp = np.zeros((96, 81), np.float32)
    # res_trans = leaky(W2 @ (W1 @ t)) = leaky(Weff @ t); /9 per side gives
    # the att /81 overall (leaky is positively homogeneous). Rows 81:96 are
    # zero so the K=96 pad contributes nothing.
    weffp[:81] = (W2 @ W1 / 9.0).T
    common = {
        "lwx": make_lw(Wx, bx),
        "lwy": make_lw(Wy, by),
        "weffp": weffp.astype(BF_NP),
    }
    return [
        {"xs3": xs_l[j], "ys3": ys_l[j], **common} for j in range(N_CORES)
    ]


_CACHED_NC = None


def get_program():
    global _CACHED_NC
    if _CACHED_NC is None:
        _CACHED_NC = build_program()
    return _CACHED_NC


def _probe_device():
    """Absorb a wedged-worker state left by a previous process: the first
    device op after a wedge fails and resets the worker; a retry succeeds."""
    import time

    import jax

    for _ in range(3):
        try:
            jax.block_until_ready(
                jax.jit(lambda a: a + 1)(np.zeros(8, np.float32))
            )
            return
        except Exception:
            time.sleep(2)


def kernel(x, y, Wx, bx, Wy, by, W1, W2):
    import time

    nc = get_program()
    in_maps = host_inputs(x, y, Wx, bx, Wy, by, W1, W2)
    _probe_device()
    last = None
    for _ in range(2):
        try:
            res = bass_utils.run_bass_kernel_spmd(
                nc, in_maps, core_ids=list(range(N_CORES))
            )
            break
        except Exception as e:
            last = e
            time.sleep(2)
    else:
        raise last
    out = np.concatenate([r["att"] for r in res.results], axis=0)[None]
    return out.astype(np.float32)
